# revision 2
# baseline (speedup 1.0000x reference)
"""MLA prefill kernel for TRN2, 8 NeuronCores — DMA-batched + S^T attention.

Sharding (as baseline): data-parallel over 128-row query blocks. Flattened
rows are [B*S] = 4096 = 2 batches x 16 blocks of 128. Core c (batch b=c//4,
j=c%4) owns blocks {j, 7-j, 8+j, 15-j} of its batch; K^T/V are AllGathered
within each batch group of 4 cores.

Design vs baseline:
- Few large DMAs (HWDGE fixed cost ~625ns per dma_start instruction).
- Attention computed transposed: S^T = K Q^T with keys on the partition dim,
  exp reads PSUM directly, and P V runs as V^T P^T with V in natural layout
  (no per-tile PE transposes, no DVE transpose evictions). Row sums via
  ones-vector matmuls; per-head normalization via a k=1 broadcast matmul.
- Union causal schedule over key blocks (identical program on all cores):
  for key block i only q-slots i//4..3 are computed; a host-built additive
  mask on the entry slot supplies per-core exactness (visible / diagonal /
  absent).
- Scheduling: Q down-projection runs on PE while the KV LayerNorm chain is
  on Act/DVE; down-proj activations staged bf16; attention inner loop is
  software-pipelined two key-blocks deep; W_o chunks prefetch during
  attention.
"""

import math

import numpy as np
import ml_dtypes

import concourse.bass as bass
import concourse.tile as tile
import concourse.mybir as mybir
from concourse import bacc
from concourse.bass_utils import run_bass_kernel_spmd

BF16 = mybir.dt.bfloat16
F32 = mybir.dt.float32
NP_BF16 = ml_dtypes.bfloat16

B, S, D = 2, 2048, 2048
H, DH = 16, 128
P = 128
NCORES = 8
RPC = 512
ROPE_THETA = 10000.0
LN_EPS = 1e-5
NEG = -30000.0
V_OFF = H * RPC            # 8192
KV_COLS = 2 * V_OFF        # 16384

AF = mybir.ActivationFunctionType
ALU = mybir.AluOpType


def _blocks(c):
    j = c % 4
    return [j, 7 - j, 8 + j, 15 - j]


def _rank_slot(i):
    """Batch-local key block i (0..15) -> (rank offset in group, slot)."""
    if i < 4:
        return i, 0
    if i < 8:
        return 7 - i, 1
    if i < 12:
        return i - 8, 2
    return 15 - i, 3


# ---------------------------------------------------------------- emission


def _emit(nc, tc, t_in, t_out):
    xt_d = t_in["xt"].ap()
    wdq = t_in["wdq"].ap()
    wuq = t_in["wuq"].ap()
    wdkv = t_in["wdkv"].ap()
    wukv = t_in["wukv"].ap()
    wot = t_in["wot"].ap()
    gb_d = t_in["gb"].ap()
    cs_d = t_in["cs"].ap()
    masks_d = t_in["masks"].ap()
    out_d = t_out["out"].ap()
    ckv_d = t_out["ckv"].ap()

    import os as _os
    no_cc = bool(_os.environ.get("BASS_MLA_NO_CC"))

    with (
        tc.tile_pool(name="big", bufs=1) as big,
        tc.tile_pool(name="wp", bufs=2) as wp,
        tc.tile_pool(name="stat", bufs=8) as stat,
        tc.tile_pool(name="rp", bufs=1) as rp,
        tc.tile_pool(name="dram", bufs=1, space="DRAM") as dram,
    ):
        qT = big.tile([P, H, RPC], BF16, tag="qT")
        oT = big.tile([P, H, RPC], BF16, tag="oT")
        ones_m = big.tile([P, 1], BF16, tag="ones_m")
        ones_k = big.tile([1, P], BF16, tag="ones_k")
        nc.vector.memset(ones_m[:], 1.0)
        nc.vector.memset(ones_k[:], 1.0)

        kv_in = dram.tile([P, KV_COLS], BF16)
        kv_out = dram.tile([4 * P, KV_COLS], BF16)

        def w_chunk(wd, c0, split=False, between=None):
            w = wp.tile([P, 16, 512], BF16, tag="w")
            src = wd.rearrange("(kt p) n -> p kt n", p=P)
            if split:
                nc.sync.dma_start(w[:, 0:8, :], src[:, 0:8, c0 : c0 + 512])
                if between is not None:
                    between()
                nc.sync.dma_start(w[:, 8:16, :], src[:, 8:16, c0 : c0 + 512])
            else:
                nc.sync.dma_start(w[:], src[:, :, c0 : c0 + 512])
            return w

        with (
            tc.tile_pool(name="front", bufs=1) as front,
            tc.tile_pool(name="ds", bufs=1) as ds,
            tc.tile_pool(name="rope", bufs=1) as rope,
            tc.tile_pool(name="k4p", bufs=2) as k4p,
            tc.tile_pool(name="kbfp", bufs=1) as kbfp,
            tc.tile_pool(name="mm", bufs=8, space="PSUM") as mm,
        ):
            xT = front.tile([P, 16, RPC], BF16, tag="xT")
            cs = front.tile([P, 4, RPC], F32, tag="cs")
            xt_v = xt_d.rearrange("p (kt n) -> p kt n", kt=16)
            nc.sync.dma_start(xT[:, 0:8, :], xt_v[:, 0:8, :])

            def xt_rest():
                nc.sync.dma_start(xT[:, 8:16, :], xt_v[:, 8:16, :])

            def down_mm(wd, raw_tag, first):
                """x @ W -> bf16 raw staging + per-chunk row sums (f32)."""
                raw = ds.tile([P, 4, D], BF16, tag=raw_tag)
                pps = {}
                for cc in range(4):
                    w = w_chunk(
                        wd, cc * 512,
                        split=(first and cc == 0),
                        between=(xt_rest if first and cc == 0 else None),
                    )
                    for rt in range(4):
                        ps = mm.tile([P, 512], F32)
                        for kt in range(16):
                            nc.tensor.matmul(
                                ps,
                                xT[:, kt, rt * P : (rt + 1) * P],
                                w[:, kt, :],
                                start=(kt == 0),
                                stop=(kt == 15),
                            )
                        pp = stat.tile([P, 1], F32, tag=f"pp_{raw_tag}{rt}{cc}")
                        nc.scalar.activation(
                            raw[:, rt, cc * 512 : (cc + 1) * 512],
                            ps,
                            AF.Copy,
                            accum_out=pp,
                        )
                        pps[(rt, cc)] = pp
                return raw, pps

            def ln_part(raw, pps, gb_off, actT, ckv_dma):
                # LN output (bf16) overwrites the raw staging rows in place;
                # each row is then transposed into actT via the DMA xbar.
                gbt = front.tile([P, 2, D], BF16, tag="gb")
                nc.scalar.dma_start(
                    gbt[:],
                    gb_d.rearrange("p (f n) -> p f n", f=4)[
                        :, gb_off : gb_off + 2, :
                    ],
                )
                gsl = gbt[:, 0, :]
                bsl = gbt[:, 1, :]
                for rt in range(4):
                    row = raw[:, rt, :]
                    s01 = stat.tile([P, 1], F32, tag="s")
                    s23 = stat.tile([P, 1], F32, tag="s")
                    ssum = stat.tile([P, 1], F32, tag="s")
                    nc.vector.tensor_tensor(s01, pps[(rt, 0)], pps[(rt, 1)], ALU.add)
                    nc.vector.tensor_tensor(s23, pps[(rt, 2)], pps[(rt, 3)], ALU.add)
                    nc.vector.tensor_tensor(ssum, s01, s23, ALU.add)
                    nmu = stat.tile([P, 1], F32, tag="s")
                    nc.vector.tensor_scalar_mul(nmu, ssum, -1.0 / D)
                    lns = ds.tile([P, D], F32, tag="lns")
                    ssq = stat.tile([P, 1], F32, tag="s")
                    nc.scalar.activation(lns, row, AF.Square, bias=nmu, accum_out=ssq)
                    veps = stat.tile([P, 1], F32, tag="s")
                    nc.vector.tensor_scalar(
                        veps, ssq, 1.0 / D, LN_EPS, ALU.mult, ALU.add
                    )
                    std = stat.tile([P, 1], F32, tag="s")
                    nc.scalar.activation(std, veps, AF.Sqrt)
                    rstd = stat.tile([P, 1], F32, tag="s")
                    nc.vector.reciprocal(rstd, std)
                    nmr = stat.tile([P, 1], F32, tag="s")
                    nc.vector.tensor_tensor(nmr, nmu, rstd, ALU.mult)
                    lns2 = ds.tile([P, D], F32, tag="lns")
                    nc.scalar.activation(lns2, row, AF.Identity, bias=nmr, scale=rstd)
                    nc.gpsimd.tensor_tensor(lns2, lns2, gsl, ALU.mult)
                    if ckv_dma:
                        nc.gpsimd.tensor_tensor(lns2, lns2, bsl, ALU.add)
                        nc.scalar.dma_start(
                            ckv_d[rt * P : (rt + 1) * P, :], lns2[:]
                        )
                        nc.scalar.activation(row, lns2, AF.Copy)
                    else:
                        nc.gpsimd.tensor_tensor(row, lns2, bsl, ALU.add)
                    nc.scalar.dma_start_transpose(actT[:, :, rt, :], row)

            def up_rope(wu, col0, actT, cos_sl, sin_sl, dst_fn):
                """4 groups of 4 heads: up-proj -> RoPE -> dst_fn(g)."""
                cos_b = cos_sl.rearrange("p (o n) -> p o n", o=1).broadcast_to(
                    [P, 4, RPC]
                )
                sin_b = sin_sl.rearrange("p (o n) -> p o n", o=1).broadcast_to(
                    [P, 4, RPC]
                )
                flush = [None]

                def do_flush():
                    if flush[0] is not None:
                        flush[0]()
                        flush[0] = None

                for g in range(4):
                    w = w_chunk(wu, col0 + g * 512)
                    do_flush()
                    k4 = k4p.tile([P, 4, RPC], BF16, tag="k4")
                    for hh in range(4):
                        ps = mm.tile([P, RPC], F32)
                        for kt in range(16):
                            nc.tensor.matmul(
                                ps,
                                w[:, kt, hh * P : (hh + 1) * P],
                                actT[:, kt, :, :],
                                start=(kt == 0),
                                stop=(kt == 15),
                            )
                        nc.scalar.activation(k4[:, hh, :], ps, AF.Copy)
                    rot = rope.tile([P, 4, RPC], BF16, tag="rot")
                    nc.scalar.dma_start(rot[0:64, :, :], k4[64:128, :, :])
                    nc.scalar.dma_start(rot[64:128, :, :], k4[0:64, :, :])
                    t2 = rope.tile([P, 4, RPC], BF16, tag="t2")
                    nc.vector.tensor_tensor(t2[:], rot[:], sin_b, ALU.mult)
                    acc = rope.tile([P, 4, RPC], BF16, tag="rot")
                    nc.vector.tensor_tensor(acc[:], k4[:], cos_b, ALU.mult)
                    flush[0] = dst_fn(g, acc, t2)
                do_flush()

            # ---- phase 1: both down-projections (PE), KV LN on Act/Pool --
            kv_raw, kv_pps = down_mm(wdkv, "kvraw", first=True)
            ckvT = ds.tile([P, 16, 4, P], BF16, tag="actT")
            ln_part(kv_raw, kv_pps, 2, ckvT, ckv_dma=True)
            q_raw, q_pps = down_mm(wdq, "qraw", first=False)

            # ---- K up-proj + rope -> kv_in ----
            def k_dst(g, acc, t2):
                kbf = kbfp.tile([P, 4, RPC], BF16, tag="kbf")
                nc.vector.tensor_tensor(kbf[:], acc[:], t2[:], ALU.add)

                def fl():
                    nc.sync.dma_start(
                        kv_in[:, g * 2048 : (g + 1) * 2048], kbf[:]
                    )
                    if no_cc:
                        for r in range(4):
                            nc.gpsimd.dma_start(
                                kv_out[r * P : (r + 1) * P,
                                       g * 2048 : (g + 1) * 2048],
                                kv_in[:, g * 2048 : (g + 1) * 2048],
                            )
                return fl

            nc.sync.dma_start(cs[:], cs_d.rearrange("p (f n) -> p f n", f=4))
            up_rope(wukv, 0, ckvT, cs[:, 0, :], cs[:, 1, :], k_dst)

            # ---- Q LN (Act/DVE, overlaps K up-proj on PE) ----
            cqT = ds.tile([P, 16, 4, P], BF16, tag="actT2")
            ln_part(q_raw, q_pps, 0, cqT, ckv_dma=False)

            # ---- V: natural layout, staged, head-major DRAM layout ----
            # (reuses the KV raw buffer, dead after the KV transposes)
            v_st = ds.tile([P, 4, D], BF16, tag="kvraw")
            for cc in range(4):
                w = w_chunk(wukv, D + cc * 512)
                for sl in range(4):
                    ps = mm.tile([P, 512], F32)
                    for kt in range(16):
                        nc.tensor.matmul(
                            ps,
                            ckvT[:, kt, sl, :],
                            w[:, kt, :],
                            start=(kt == 0),
                            stop=(kt == 15),
                        )
                    nc.vector.tensor_copy(
                        v_st[:, sl, cc * 512 : (cc + 1) * 512], ps
                    )
            kvi_v = kv_in[:, V_OFF:KV_COLS].rearrange(
                "p (hh sl dd) -> p hh sl dd", hh=H, sl=4, dd=P
            )
            for sl in range(4):
                nc.sync.dma_start(
                    kvi_v[:, :, sl, :],
                    v_st[:, sl, :].rearrange("p (hh dd) -> p hh dd", hh=H),
                )
            if no_cc:
                for r in range(4):
                    nc.gpsimd.dma_start(
                        kv_out[r * P : (r + 1) * P, V_OFF:KV_COLS],
                        kv_in[:, V_OFF:KV_COLS],
                    )

            # Real collective goes out as early as possible (gpsimd queue,
            # which nothing below uses); the sim fallback is emitted late so
            # its DMA-engine traffic doesn't starve the Q-path weight loads.
            if not no_cc:
                nc.gpsimd.collective_compute(
                    "AllGather",
                    ALU.bypass,
                    replica_groups=[[0, 1, 2, 3], [4, 5, 6, 7]],
                    ins=[kv_in.opt()],
                    outs=[kv_out.opt()],
                )

            # ---- Q up-proj + rope -> qT ----
            def q_dst(g, acc, t2):
                nc.vector.tensor_tensor(
                    qT[:, 4 * g : 4 * g + 4, :], acc[:], t2[:], ALU.add
                )
                return None

            up_rope(wuq, 0, cqT, cs[:, 2, :], cs[:, 3, :], q_dst)

        # ================= attention =================
        kvK = kv_out.rearrange(
            "(ro p) (half hh sl kk) -> p ro half hh sl kk",
            p=P, half=2, hh=H, sl=4, kk=P,
        )
        kvV = kv_out.rearrange(
            "(ro p) (half hh sl dd) -> p ro half hh sl dd",
            p=P, half=2, hh=H, sl=4, dd=P,
        )
        with (
            tc.tile_pool(name="att", bufs=2) as att,
            tc.tile_pool(name="pbp", bufs=4) as pbp,
            tc.tile_pool(name="mkp", bufs=1) as mkp,
            tc.tile_pool(name="scp", bufs=3, space="PSUM") as scp,
            tc.tile_pool(name="otp", bufs=2, space="PSUM") as otp,
            tc.tile_pool(name="lsp", bufs=2, space="PSUM") as lsp,
            tc.tile_pool(name="rbp", bufs=1, space="PSUM") as rbp,
        ):
            # 0/1 multiplicative masks: applied to exp output on the Pool
            # engine, entry slot only (visible=1 / diagonal tri / absent=0).
            masks = mkp.tile([P, 16, P], BF16, tag="masks")
            nc.sync.dma_start(
                masks[:], masks_d.rearrange("p (i n) -> p i n", i=16)
            )
            wot_pre = [None, None]

            for h in range(H):
                kt_t = att.tile([P, 4, RPC], BF16, tag="kt")
                v_t = att.tile([P, 4, 4, P], BF16, tag="v")
                nc.sync.dma_start(kt_t[:], kvK[:, :, 0, h, :, :])
                nc.sync.dma_start(v_t[:], kvV[:, :, 1, h, :, :])
                if h == 0:
                    wot_pre[0] = w_chunk(wot, 0)
                    wot_pre[1] = w_chunk(wot, 512)
                # One accumulation group per PSUM bank: start=True zeroes the
                # whole 2KB bank, so each of oT/ls gets exactly one start (at
                # block 0, full width) and shrinking-suffix accumulation.
                oT_ps = otp.tile([P, RPC], F32)
                ls_ps = lsp.tile([1, RPC], F32)

                def sc_exp(i):
                    ro, sl = _rank_slot(i)
                    qs = i // 4
                    n0 = qs * P
                    ps = scp.tile([P, RPC], F32)
                    nc.tensor.matmul(
                        ps[:, n0:RPC],
                        kt_t[:, ro, sl * P : (sl + 1) * P],
                        qT[:, h, n0:RPC],
                        start=True,
                        stop=True,
                    )
                    pb = pbp.tile([P, RPC], BF16, tag="pb")
                    nc.scalar.activation(pb[:, n0:RPC], ps[:, n0:RPC], AF.Exp)
                    nc.gpsimd.tensor_tensor(
                        pb[:, n0 : n0 + P],
                        pb[:, n0 : n0 + P],
                        masks[:, i, :],
                        ALU.mult,
                    )
                    return pb

                def av_ls(i, pb):
                    ro, sl = _rank_slot(i)
                    n0 = (i // 4) * P
                    nc.tensor.matmul(
                        oT_ps[:, n0:RPC],
                        v_t[:, ro, sl, :],
                        pb[:, n0:RPC],
                        start=(i == 0),
                        stop=(i == 15),
                        skip_group_check=True,
                    )
                    nc.tensor.matmul(
                        ls_ps[0:1, n0:RPC],
                        ones_m[:],
                        pb[:, n0:RPC],
                        start=(i == 0),
                        stop=(i == 15),
                        skip_group_check=True,
                    )

                pbs = {}
                for i in range(18):
                    if i < 16:
                        pbs[i] = sc_exp(i)
                    if i >= 2:
                        av_ls(i - 2, pbs.pop(i - 2))

                r_bf = rp.tile([1, RPC], BF16, tag="rbf")
                with nc.allow_low_precision(reason="softmax denom"):
                    nc.vector.reciprocal(r_bf[:], ls_ps)
                if "dbg_ls" in t_out:
                    ls_sb = mkp.tile([1, RPC], F32, tag="lssb")
                    nc.vector.tensor_copy(ls_sb[:], ls_ps)
                    nc.sync.dma_start(
                        t_out["dbg_ls"].ap()[h : h + 1, :], ls_sb[:]
                    )
                rb_ps = rbp.tile([P, RPC], F32)
                nc.tensor.matmul(rb_ps, ones_k[:], r_bf[:], start=True, stop=True)
                rb_sb = rp.tile([P, RPC], F32, tag="rbsb")
                nc.scalar.activation(rb_sb[:], rb_ps, AF.Copy)
                nc.vector.tensor_tensor(oT[:, h, :], oT_ps, rb_sb[:], ALU.mult)

        # ================= output projection =================
        with (
            tc.tile_pool(name="ost", bufs=1) as ost,
            tc.tile_pool(name="mm4", bufs=4, space="PSUM") as mm4,
        ):
            o_st = ost.tile([P, 4, D], F32, tag="ost")
            for cc in range(4):
                w = wot_pre[cc] if cc < 2 else w_chunk(wot, cc * 512)
                for rt in range(4):
                    ps = mm4.tile([P, 512], F32)
                    for kt in range(16):
                        nc.tensor.matmul(
                            ps,
                            oT[:, kt, rt * P : (rt + 1) * P],
                            w[:, kt, :],
                            start=(kt == 0),
                            stop=(kt == 15),
                        )
                    dsto = o_st[:, rt, cc * 512 : (cc + 1) * 512]
                    if (cc + rt) % 2 == 0:
                        nc.vector.tensor_copy(dsto, ps)
                    else:
                        nc.scalar.activation(dsto, ps, AF.Copy)
                nc.sync.dma_start(
                    out_d.rearrange("(rt p) d -> p rt d", p=P)[
                        :, :, cc * 512 : (cc + 1) * 512
                    ],
                    o_st[:, :, cc * 512 : (cc + 1) * 512],
                )


# ---------------------------------------------------------------- build


_CACHE = {}


def _build():
    if "nc" in _CACHE:
        return _CACHE["nc"]
    nc = bacc.Bacc("TRN2", target_bir_lowering=False, debug=False, num_devices=NCORES)
    t_in = {}

    def inp(name, shape, dt):
        t_in[name] = nc.dram_tensor(name, shape, dt, kind="ExternalInput")

    inp("xt", [P, 16 * RPC], BF16)
    inp("wdq", [D, D], BF16)
    inp("wuq", [D, D], BF16)
    inp("wdkv", [D, D], BF16)
    inp("wukv", [D, 2 * D], BF16)
    inp("wot", [D, D], BF16)
    inp("gb", [P, 4 * D], BF16)
    inp("cs", [P, 4 * RPC], F32)
    inp("masks", [P, 16 * P], BF16)
    t_out = {
        "out": nc.dram_tensor("out", [RPC, D], F32, kind="ExternalOutput"),
        "ckv": nc.dram_tensor("ckv", [RPC, D], F32, kind="ExternalOutput"),
    }
    import os as _os
    if _os.environ.get("BASS_MLA_DEBUG"):
        t_out["dbg_ls"] = nc.dram_tensor(
            "dbg_ls", [H, RPC], F32, kind="ExternalOutput"
        )
    with tile.TileContext(nc) as tc:
        _emit(nc, tc, t_in, t_out)
    nc.finalize()
    _CACHE["nc"] = nc
    return nc


# ---------------------------------------------------------------- host


def host_prep(inputs):
    x = np.asarray(inputs["x"], np.float32).reshape(B * S, D)
    wdq_ = np.asarray(inputs["W_dq"], np.float32).astype(NP_BF16)
    wuq_ = np.asarray(inputs["W_uq"], np.float32).astype(NP_BF16)
    wdkv_ = np.asarray(inputs["W_dkv"], np.float32).astype(NP_BF16)
    wukv_ = np.asarray(inputs["W_ukv"], np.float32).astype(NP_BF16)
    wot_ = np.ascontiguousarray(np.asarray(inputs["W_o"], np.float32).T).astype(
        NP_BF16
    )

    def bc(v):
        return np.broadcast_to(np.asarray(v, np.float32), (P, D))

    gb = np.concatenate(
        [bc(inputs["q_gamma"]), bc(inputs["q_beta"]),
         bc(inputs["kv_gamma"]), bc(inputs["kv_beta"])], axis=1
    ).astype(NP_BF16)
    gb = np.ascontiguousarray(gb)

    freqs = 1.0 / (ROPE_THETA ** (np.arange(0, DH, 2, dtype=np.float32) / DH))
    t = np.arange(S, dtype=np.float32)
    emb = np.outer(t, freqs)
    cos = np.concatenate([np.cos(emb), np.cos(emb)], -1).T.astype(np.float32)
    sin = np.concatenate([np.sin(emb), np.sin(emb)], -1).T.astype(np.float32)
    sin_signed = sin.copy()
    sin_signed[:64] *= -1.0
    scale = 1.0 / math.sqrt(DH)

    # S^T-layout 0/1 diagonal mask [key kk, q qq]: visible iff kk <= qq.
    tri = (
        np.arange(P)[:, None] <= np.arange(P)[None, :]
    ).astype(np.float32)

    in_maps = []
    for c in range(NCORES):
        b = c // 4
        blks = _blocks(c)
        rows = np.concatenate([np.arange(bl * P, (bl + 1) * P) for bl in blks])
        x_c = np.ascontiguousarray(x[b * S + rows])  # [512, D]
        xt = np.ascontiguousarray(
            x_c.T.reshape(16, P, RPC).transpose(1, 0, 2).reshape(P, 16 * RPC)
        ).astype(NP_BF16)

        cs_c = np.ascontiguousarray(
            np.concatenate(
                [cos[:, rows], sin_signed[:, rows],
                 cos[:, rows] * scale, sin_signed[:, rows] * scale], axis=1
            )
        ).astype(np.float32)

        m = np.zeros((P, 16, P), np.float32)
        for i in range(16):
            blk_e = blks[i // 4]
            if i == blk_e:
                m[:, i, :] = tri
            elif i < blk_e:
                m[:, i, :] = 1.0
        masks = np.ascontiguousarray(m.reshape(P, 16 * P)).astype(NP_BF16)

        in_maps.append(
            {
                "xt": xt,
                "wdq": wdq_, "wuq": wuq_, "wdkv": wdkv_, "wukv": wukv_,
                "wot": wot_,
                "gb": gb,
                "cs": cs_c,
                "masks": masks,
            }
        )
    return in_maps


def host_unshard(results):
    out = np.zeros((B * S, D), np.float32)
    ckv = np.zeros((B * S, D), np.float32)
    for c in range(NCORES):
        b = c // 4
        for qs, blk in enumerate(_blocks(c)):
            g = b * S + blk * P
            out[g : g + P] = results[c]["out"][qs * P : (qs + 1) * P]
            ckv[g : g + P] = results[c]["ckv"][qs * P : (qs + 1) * P]
    return out.reshape(B, S, D), ckv.reshape(B, S, D)


def kernel(**inputs):
    nc = _build()
    in_maps = host_prep(inputs)
    res = run_bass_kernel_spmd(nc, in_maps, core_ids=list(range(NCORES)))
    return host_unshard(res.results)


if __name__ == "__main__":
    rng = np.random.default_rng(0)
    ins = {
        "x": rng.standard_normal((B, S, D), np.float32),
        "W_dq": 0.02 * rng.standard_normal((D, D), np.float32),
        "W_uq": 0.02 * rng.standard_normal((D, D), np.float32),
        "q_gamma": np.ones(D, np.float32),
        "q_beta": np.zeros(D, np.float32),
        "W_dkv": 0.02 * rng.standard_normal((D, D), np.float32),
        "W_ukv": 0.02 * rng.standard_normal((D, 2 * D), np.float32),
        "kv_gamma": np.ones(D, np.float32),
        "kv_beta": np.zeros(D, np.float32),
        "W_o": 0.02 * rng.standard_normal((D, D), np.float32),
    }
    o, ck = kernel(**ins)
    print(o.shape, ck.shape, float(np.abs(o).mean()), float(np.abs(ck).mean()))
